# revision 19
# baseline (speedup 1.0000x reference)
"""DiT attention kernel for 8 Trainium2 NeuronCores.

Sharding: data-parallel over batch (B=2 -> core groups {0-3}, {4-7}) x
tensor-parallel over heads (16 heads -> 4 per core). Each core computes its
4 heads' attention and partial output projections (its rows of w_out), one
output tensor per head-pair; host sums the 8 partials per batch element.

Per-core device pipeline (all matmuls bf16 inputs, fp32 PSUM accumulate):
  MM0v : v = x @ wv (natural [T, d] layout, ones-column interleaved)
  MM0qk: qT/kT = wq^T-style projection producing [d, T] tiles, q pre-scaled
         by HD^-0.5 on host; RoPE (head 0 only) applied via replicated
         cos/sin tables that are identity on non-RoPE cores.
  MM1  : scores^T[j, i] = kT.T @ qT per head (softmax j on partitions)
  exp  : ACT Exp with per-partition mask bias (masked j -> exp 0)
  MM2  : out_raw^T[d+1, i] accumulated over j-tiles; lhsT = [v_h | ones]
         so row 64 is the softmax denominator (rowsum)
  norm : out^T = out_raw^T * (1/rowsum) broadcast via DRAM round-trip
  MM3  : partial = out^T.T @ wo_rows
"""
import numpy as np
import ml_dtypes

B, T, DIM = 2, 2048, 1024
H, HD = 16, 64
HPC = 4            # heads per core
NCORES = 8
KT = DIM // 128    # 8 contraction k-tiles
JT = T // 128      # 16 j-tiles
ROPE_BASE = 10000.0
BF16 = ml_dtypes.bfloat16

_CACHE = {}


def _split_waits(nc, mybir, max_waits=1):
    """Hoist extra sem-waits onto standalone NoOps: this walrus build allows
    only one sync-wait slot per TPB instruction."""
    for f in nc.m.functions:
        for b in f.blocks:
            new = []
            for inst in b.instructions:
                tname = type(inst).__name__
                si = inst.sync_info
                if (
                    tname != "InstEventSemaphore"
                    and si is not None
                    and si.on_wait
                    and len(si.on_wait) > max_waits
                ):
                    waits = list(si.on_wait)
                    for i, w in enumerate(waits[max_waits:]):
                        nop = mybir.InstNoOp(name=f"{inst.name}-wsplit{i}", ins=[], outs=[])
                        nop.engine = inst.engine
                        nop.sync_info = mybir.SyncInfo(on_wait=[w], on_update=[])
                        new.append(nop)
                    inst.sync_info = mybir.SyncInfo(
                        on_wait=waits[:max_waits], on_update=list(si.on_update or [])
                    )
                new.append(inst)
            b.instructions = new


def _build():
    import concourse.bass as bass
    import concourse.mybir as mybir
    import concourse.tile as tile

    F32 = mybir.dt.float32
    B16 = mybir.dt.bfloat16
    AF = mybir.ActivationFunctionType

    nc = bass.Bass()
    xT = nc.dram_tensor("xT", [DIM, T], B16, kind="ExternalInput")
    wqk = nc.dram_tensor("wqk", [DIM, 512], B16, kind="ExternalInput")
    wv = nc.dram_tensor("wv", [DIM, 260], B16, kind="ExternalInput")
    wo = nc.dram_tensor("wo", [256, DIM], B16, kind="ExternalInput")
    cosq = nc.dram_tensor("cosq", [128, T], F32, kind="ExternalInput")
    sinq = nc.dram_tensor("sinq", [128, T], F32, kind="ExternalInput")
    mb = nc.dram_tensor("mb", [128, JT], F32, kind="ExternalInput")
    out_a = nc.dram_tensor("out_a", [T, DIM], F32, kind="ExternalOutput")
    out_b = nc.dram_tensor("out_b", [T, DIM], F32, kind="ExternalOutput")

    with tile.TileContext(nc) as tc:
        with tc.tile_pool(name="persist", bufs=1) as persist, \
             tc.tile_pool(name="work", bufs=2) as work, \
             tc.tile_pool(name="expp", bufs=6) as expp, \
             tc.tile_pool(name="nrmp", bufs=3) as nrmp, \
             tc.tile_pool(name="outp", bufs=4) as outp, \
             tc.tile_pool(name="dramp", bufs=2, space="DRAM") as dramp, \
             tc.tile_pool(name="ps", bufs=1, space="PSUM") as ps:

            # ---- persistent loads (weights first so MM0 starts ASAP) ----
            wqk_sb = persist.tile([128, KT, 512], B16)
            nc.sync.dma_start(wqk_sb, wqk[:, :].rearrange("(kt p) c -> p kt c", p=128))
            wv_sb = persist.tile([128, KT, 260], B16)
            nc.sync.dma_start(wv_sb, wv[:, :].rearrange("(kt p) c -> p kt c", p=128))
            xt = persist.tile([128, KT, T], B16)
            for kt in range(KT):
                nc.sync.dma_start(xt[:, kt, :], xT[kt * 128:(kt + 1) * 128, :])
            cos_sb = persist.tile([128, T], mybir.dt.float32)
            nc.sync.dma_start(cos_sb, cosq[:, :])
            sin_sb = persist.tile([128, T], mybir.dt.float32)
            nc.sync.dma_start(sin_sb, sinq[:, :])
            mb_sb = persist.tile([128, JT], mybir.dt.float32)
            nc.sync.dma_start(mb_sb, mb[:, :])
            wo_sb = persist.tile([128, 2, DIM], B16)
            nc.sync.dma_start(wo_sb, wo[:, :].rearrange("(pr p) c -> p pr c", p=128))

            v_sb = persist.tile([128, JT, 260], B16)
            qpair = persist.tile([128, 2, T], B16)
            kpair = persist.tile([128, 2, T], B16)
            outT = persist.tile([128, 2, T], B16)

            # warm the ACT exp table set during the projection phase so the
            # ~2.7us table load isn't on the first real exp's critical path
            warm = persist.tile([1, 1], mybir.dt.float32)
            nc.vector.memset(warm, 0.0)
            nc.scalar.activation(warm, warm, AF.Exp, bias=0.0, scale=1.0)

            # ---- MM0v: v in natural [j, d] layout, 4 heads interleaved at 65 ----
            v_view = v_sb.rearrange("p j (g r) -> p j g r", r=65)

            def mm0v(jt):
                pv = ps.tile([128, 512], mybir.dt.float32, tag="mm", bufs=2, name="pv")
                for kt in range(KT):
                    nc.tensor.matmul(
                        pv[:, 0:260],
                        xt[:, kt, jt * 128:(jt + 1) * 128],
                        wv_sb[:, kt, :],
                        start=(kt == 0), stop=(kt == KT - 1),
                    )
                nc.vector.tensor_copy(v_sb[:, jt, :], pv[:, 0:260])
                # ones column (col 64 of each 65-wide head group)
                nc.gpsimd.memset(v_view[:, jt, :, 64:65], 1.0)

            # ---- MM0qk: ct 0=q01(rope) 1=q23 2=k01(rope) 3=k23 ----
            def mm0qk(ct):
                dest = (qpair if ct in (0, 1) else kpair)[:, ct % 2, :]
                ropey = ct % 2 == 0
                if ropey:
                    stage = work.tile([128, T], mybir.dt.float32, tag="ropestage", name="stage")
                    sw = work.tile([128, T], mybir.dt.float32, tag="sw", name="sw")
                for ts in range(4):
                    s = slice(ts * 512, (ts + 1) * 512)
                    pqk = ps.tile([128, 512], mybir.dt.float32, tag="mm", bufs=2, name="pqk")
                    for kt in range(KT):
                        nc.tensor.matmul(
                            pqk,
                            wqk_sb[:, kt, ct * 128:(ct + 1) * 128],
                            xt[:, kt, s],
                            start=(kt == 0), stop=(kt == KT - 1),
                        )
                    if ropey:
                        # q_r = q*cos + swap(q)*sinA, chunked per 512 cols so
                        # the rope chain overlaps the remaining projections.
                        # Rows 64:128 are the pair's other head (identity:
                        # cos rows 64:128 == 1, sin rows == 0).
                        nc.vector.tensor_copy(stage[:, s], pqk)
                        nc.gpsimd.memset(sw[64:128, s], 0.0)
                        nc.gpsimd.tensor_copy(sw[0:32, s], stage[32:64, s])
                        nc.gpsimd.tensor_copy(sw[32:64, s], stage[0:32, s])
                        nc.vector.tensor_mul(stage[:, s], stage[:, s], cos_sb[:, s])
                        nc.vector.tensor_mul(sw[:, s], sw[:, s], sin_sb[:, s])
                        nc.vector.tensor_add(dest[:, s], stage[:, s], sw[:, s])
                    else:
                        nc.vector.tensor_copy(dest[:, s], pqk)

            # ---- attention: one head-pair, one 512-wide i-quarter at a time.
            # MM1 runs both heads concurrently (row groups 0-63 / 64-127 via
            # base-partition-derived tile_position); one Exp instruction
            # covers both heads' scores (the mask bias is per-j, shared).
            def attn_iq(pr, iq):
                i0 = iq * 512
                accs = [
                    ps.tile([65, 512], mybir.dt.float32, tag=f"acc{s}", bufs=1,
                            name=f"acc{s}")
                    for s in range(2)
                ]
                for jt in range(JT):
                    sps = ps.tile([128, 1024], mybir.dt.float32, tag="scores", bufs=2, name="sps")
                    for s in range(2):
                        b = s * 64
                        nc.tensor.matmul(
                            sps[:, s * 512:(s + 1) * 512],
                            kpair[b:b + 64, pr, jt * 128:(jt + 1) * 128],
                            qpair[b:b + 64, pr, i0:i0 + 512],
                            start=True, stop=True,
                        )
                    e = expp.tile([128, 1024], B16, tag="exp", name="e")
                    nc.scalar.activation(e, sps, AF.Exp, bias=mb_sb[:, jt:jt + 1], scale=1.0)
                    for s in range(2):
                        h = 2 * pr + s
                        nc.tensor.matmul(
                            accs[s],
                            v_sb[:, jt, 65 * h:65 * h + 65],
                            e[:, s * 512:(s + 1) * 512],
                            start=(jt == 0), stop=(jt == JT - 1),
                        )
                # normalize: out^T rows 0:64, rowsum row 64
                for s in range(2):
                    base = s * 64
                    nrm = nrmp.tile([65, 512], mybir.dt.float32, tag="nrm", name="nrm")
                    nc.vector.tensor_copy(nrm, accs[s])
                    rr = nrmp.tile([1, 512], mybir.dt.float32, tag="rr", name="rr")
                    nc.vector.reciprocal(rr, nrm[64:65, :])
                    dr = dramp.tile([1, 512], mybir.dt.float32, name="dr")
                    nc.sync.dma_start(dr, rr)
                    rb = nrmp.tile([64, 512], mybir.dt.float32, tag="rb", name="rb")
                    nc.sync.dma_start(rb, dr[0, :].partition_broadcast(64))
                    nc.vector.tensor_mul(
                        outT[base:base + 64, pr, i0:i0 + 512],
                        nrm[0:64, :], rb,
                    )

            # ---- MM3: per-pair partial output projection; each pair gets its
            # own output tensor (host sums), so no cross-pass ordering needed
            def mm3(pr, tts):
                dst_t = out_a if pr == 0 else out_b
                for tt in tts:
                    for c in range(2):
                        po = ps.tile([128, 512], mybir.dt.float32, tag="mm", bufs=2, name="po")
                        nc.tensor.matmul(
                            po,
                            outT[:, pr, tt * 128:(tt + 1) * 128],
                            wo_sb[:, pr, c * 512:(c + 1) * 512],
                            start=True, stop=True,
                        )
                        ob = outp.tile([128, 512], mybir.dt.float32, tag="ob", name="ob")
                        if (tt + c) % 2 == 0:
                            nc.vector.tensor_copy(ob, po)
                        else:
                            nc.scalar.copy(ob, po)
                        nc.sync.dma_start(
                            dst_t[tt * 128:(tt + 1) * 128, c * 512:(c + 1) * 512], ob
                        )

            # interleave: pair-0 attention starts as soon as its q/k and v are
            # ready; pair-1 projections and pair-0's output projection fill PE
            # gaps inside the ACT-bound attention window
            mm0qk(0)
            mm0qk(2)
            for jt in range(JT):
                mm0v(jt)
            for iq in range(4):
                attn_iq(0, iq)
            mm0qk(1)
            mm0qk(3)
            attn_iq(1, 0)
            mm3(0, range(0, JT))
            attn_iq(1, 1)
            attn_iq(1, 2)
            attn_iq(1, 3)
            mm3(1, range(0, JT))

    _split_waits(nc, mybir)
    return nc


def _rope_tables():
    inv_freq = 1.0 / (ROPE_BASE ** (np.arange(0, HD, 2, dtype=np.float64) / HD))
    t = np.arange(T, dtype=np.float64)
    freqs = t[:, None] * inv_freq[None, :]            # [T, 32]
    emb = np.concatenate([freqs, freqs], axis=-1)      # [T, 64]
    return np.cos(emb).astype(np.float32), np.sin(emb).astype(np.float32)


def _prep_inputs(x, mask, w_qkv, w_out):
    x = np.asarray(x, dtype=np.float32)
    mask = np.asarray(mask)
    w_qkv = np.asarray(w_qkv, dtype=np.float32)
    w_out = np.asarray(w_out, dtype=np.float32)

    wq = w_qkv[:, 0:DIM] * (HD ** -0.5)
    wk = w_qkv[:, DIM:2 * DIM]
    wvb = w_qkv[:, 2 * DIM:3 * DIM]

    cos, sin = _rope_tables()                          # [T, 64]
    # transposed, sign-folded tables for the swap formulation, stacked for
    # head-slot 0 (rows 0:64) with identity rows 64:128
    cosq_real = np.ones((128, T), np.float32)
    cosq_real[0:64] = cos.T
    sinq_real = np.zeros((128, T), np.float32)
    sinq_real[0:32] = -sin.T[0:32]
    sinq_real[32:64] = sin.T[32:64]
    cosq_id = np.ones((128, T), np.float32)
    sinq_id = np.zeros((128, T), np.float32)

    in_maps = []
    for c in range(NCORES):
        b, hg = divmod(c, 4)
        heads = [4 * hg + i for i in range(HPC)]
        xT_c = np.ascontiguousarray(x[b].T).astype(BF16)
        qcols = np.concatenate([wq[:, HD * h:HD * (h + 1)] for h in heads], axis=1)
        kcols = np.concatenate([wk[:, HD * h:HD * (h + 1)] for h in heads], axis=1)
        wqk_c = np.ascontiguousarray(
            np.concatenate([qcols, kcols], axis=1)).astype(BF16)
        wv_c = np.zeros((DIM, 260), np.float32)
        for i, h in enumerate(heads):
            wv_c[:, 65 * i:65 * i + HD] = wvb[:, HD * h:HD * (h + 1)]
        wv_c = wv_c.astype(BF16)
        wo_c = np.ascontiguousarray(
            np.concatenate([w_out[HD * h:HD * (h + 1), :] for h in heads], axis=0)
        ).astype(BF16)
        mb_c = np.where(
            mask[b].reshape(JT, 128).T, 0.0, -30000.0
        ).astype(np.float32)                            # [128, JT]
        in_maps.append({
            "xT": xT_c,
            "wqk": wqk_c,
            "wv": wv_c,
            "wo": wo_c,
            "cosq": cosq_real if hg == 0 else cosq_id,
            "sinq": sinq_real if hg == 0 else sinq_id,
            "mb": np.ascontiguousarray(mb_c),
        })
    return in_maps


def _get_nc():
    if "nc" not in _CACHE:
        _CACHE["nc"] = _build()
    return _CACHE["nc"]


def kernel(x, mask, w_qkv, w_out, **_ignored):
    from concourse.bass_utils import run_bass_kernel_spmd

    nc = _get_nc()
    in_maps = _prep_inputs(x, mask, w_qkv, w_out)
    res = run_bass_kernel_spmd(nc, in_maps, core_ids=list(range(NCORES)))
    out = np.zeros((B, T, DIM), dtype=np.float32)
    for c in range(NCORES):
        out[c // 4] += res.results[c]["out_a"]
        out[c // 4] += res.results[c]["out_b"]
    return out
